# revision 10
# baseline (speedup 1.0000x reference)
"""TRN2 Bass kernel for nn_BilinearTensorProduct.

  out = tanh(concat(V1,V2) @ W + einsum('bd,kde,be->bk', V1, T, V2) + sum(b))
  B=8192, D=256, K=64.  Data-parallel: batch sharded 8 ways, T/W/b replicated.

Bilinear path: per (k-pair, b-tile), one PSUM group of 4 matmuls — two fp16
hi*hi chunks [128d x 128b] @ [128d x 512e] plus two fp8-e4m3 DoubleRow
correction matmuls (lo_V*hi_T and hi_V*lo_T, contraction 256 each at 0.5
cyc/row).  All operands are pre-scaled on host so every product lands in PSUM
at scale 2^22 (fp16 side: 2^11 per factor; fp8 side: the same products
rebalanced by 2^+-6 so values fit e4m3 range).  The dropped lo*lo term is
~2^-23 relative.  After the group, one fused DVE affine_mul_reduce per k
multiplies by V2 and row-reduces into the per-tile result, folding the 2^-22
unscale into its scale slot.  MM order hi,DR,hi,DR keeps every LDWEIGHTS
hidden under the preceding matmul.  The feedforward path stays a 3-pass fp16
split matmul with sum(b) folded in as an extra contraction block (ones-row in
CT, sum_b-row in W).  tanh runs on the scalar engine.
"""

import numpy as np
from contextlib import ExitStack

import concourse.bacc as bacc
import concourse.mybir as mybir
from concourse import tile
from concourse import bass_utils

B, D, K = 8192, 256, 64
NCORES = 8
BS = B // NCORES          # 1024 batch rows per core
NBT = BS // 128           # 8 b-tiles of 128 rows
NKP = K // 2              # 32 k-pairs
NC_FF = 5                 # ff contraction chunks: 4 real + 1 bias block
SCALE = 2048.0            # 2^11 pre-scale for fp16 hi/lo split
UNSCALE = 2.0 ** -22      # undo SCALE^2 after the matmul
F8S = 64.0                # fp8 rebalance factor (2^6)
F8MAX = 240.0             # e4m3 max finite; clip before cast

f32 = mybir.dt.float32
f16 = mybir.dt.float16
f8 = mybir.dt.float8e4

_NC_CACHE = {}


def _build(n_k=K):
    nc = bacc.Bacc("TRN2", target_bir_lowering=False, debug=False)
    A1 = nc.dram_tensor("A1", [D, BS], f16, kind="ExternalInput")
    A2P = nc.dram_tensor("A2P", [128, 2, BS], f8, kind="ExternalInput")
    A1P = nc.dram_tensor("A1P", [128, 2, BS], f8, kind="ExternalInput")
    THX = nc.dram_tensor("THX", [2 * NKP, 128, 512], f16, kind="ExternalInput")
    T8M = nc.dram_tensor("T8M", [NKP, 128, 2, 512], f8, kind="ExternalInput")
    TL8 = nc.dram_tensor("TL8", [NKP, 128, 2, 512], f8, kind="ExternalInput")
    V2N = nc.dram_tensor("V2N", [BS, D], f32, kind="ExternalInput")
    CTH = nc.dram_tensor("CTH", [NC_FF * 128, BS], f16, kind="ExternalInput")
    CTL = nc.dram_tensor("CTL", [NC_FF * 128, BS], f16, kind="ExternalInput")
    WH = nc.dram_tensor("WH", [NC_FF * 128, K], f16, kind="ExternalInput")
    WL = nc.dram_tensor("WL", [NC_FF * 128, K], f16, kind="ExternalInput")
    OUT = nc.dram_tensor("OUT", [BS, K], f32, kind="ExternalOutput")

    DR = mybir.MatmulPerfMode.DoubleRow

    with tile.TileContext(nc) as tc:
        with ExitStack() as ctx:
            const = ctx.enter_context(tc.tile_pool(name="const", bufs=1))
            tpool = ctx.enter_context(tc.tile_pool(name="tpool", bufs=4))
            psb = ctx.enter_context(tc.tile_pool(name="psb", bufs=7, space="PSUM"))
            psff = ctx.enter_context(tc.tile_pool(name="psff", bufs=1, space="PSUM"))
            scr = ctx.enter_context(tc.tile_pool(name="scr", bufs=3))

            a1 = [const.tile([128, BS], f16, name=f"a1_{c}", tag=f"a1_{c}")
                  for c in range(2)]
            a2p = const.tile([128, 2, BS], f8, name="a2p", tag="a2p")
            a1p = const.tile([128, 2, BS], f8, name="a1p", tag="a1p")
            v2 = [const.tile([128, D], f32, name=f"v2_{t}", tag=f"v2_{t}")
                  for t in range(NBT)]
            cth = [const.tile([128, BS], f16, name=f"cth_{c}", tag=f"cth_{c}")
                   for c in range(NC_FF)]
            ctl = [const.tile([128, BS], f16, name=f"ctl_{c}", tag=f"ctl_{c}")
                   for c in range(NC_FF)]
            wth = [const.tile([128, K], f16, name=f"wth_{c}", tag=f"wth_{c}")
                   for c in range(NC_FF)]
            wtl = [const.tile([128, K], f16, name=f"wtl_{c}", tag=f"wtl_{c}")
                   for c in range(NC_FF)]
            bil = [const.tile([128, K], f32, name=f"bil_{t}", tag=f"bil_{t}")
                   for t in range(NBT)]

            for c in range(2):
                nc.sync.dma_start(a1[c][:], A1.ap()[c * 128:(c + 1) * 128, :])

            ffp = psff.tile([128, NBT * K], f32, name="ffp", tag="ffp")

            # PE warm-up: ~3us of throwaway matmuls on a memset scratch tile
            # run during the DMA preamble, so HAM is at 8/8 and the PE is
            # draining its queue when the first real operands land.
            warm = const.tile([128, 256], f16, name="warm", tag="warm")
            nc.any.memset(warm[:], 0.0)
            # 36 x ~107ns cold > the 3.4us HAM SHORT window, so the clock
            # gate opens before the first real matmul
            for i in range(36):
                wp = psb.tile([128, 2 * D], f32, tag="ps")
                nc.tensor.matmul(wp[:, 0:128], warm[:, 0:128], warm[:, 128:256],
                                 start=True, stop=True)

            if n_k < K:  # reduced builds (sim) leave columns unwritten
                for t in range(NBT):
                    nc.any.memset(bil[t][:], 0.0)
            assert n_k % 2 == 0, "k loop is paired"
            n_kp = n_k // 2
            kp_ff = 4 if n_kp > 4 else max(n_kp - 1, 0)
            for kp in range(n_kp):
                k = 2 * kp
                # spread the 2.6MB CT/W burst over kp=1..3 so it doesn't
                # starve the T-tile stream
                if n_kp > 4:
                    ct_sched = {1: [0, 1], 2: [2, 3], 3: [4]}.get(kp, [])
                else:
                    ct_sched = list(range(NC_FF)) if kp == min(1, n_kp - 1) else []
                for c in ct_sched:
                    nc.sync.dma_start(cth[c][:], CTH.ap()[c * 128:(c + 1) * 128, :])
                    nc.sync.dma_start(ctl[c][:], CTL.ap()[c * 128:(c + 1) * 128, :])
                    nc.sync.dma_start(wth[c][:], WH.ap()[c * 128:(c + 1) * 128, :])
                    nc.sync.dma_start(wtl[c][:], WL.ap()[c * 128:(c + 1) * 128, :])
                if kp == kp_ff:
                    # feedforward (+sum(b) bias block) as fp16 hi/lo 3-pass;
                    # CT side is pre-scaled 2^11, unscaled in the epilogue
                    for t in range(NBT):
                        passes = [(cth, wth), (cth, wtl), (ctl, wth)]
                        n_mm = len(passes) * NC_FF
                        i_mm = 0
                        for cs, ws in passes:
                            for c in range(NC_FF):
                                nc.tensor.matmul(
                                    ffp[:, t * K:(t + 1) * K],
                                    cs[c][:, t * 128:(t + 1) * 128],
                                    ws[c][:],
                                    start=(i_mm == 0), stop=(i_mm == n_mm - 1),
                                )
                                i_mm += 1
                # two k's side by side: rhs [128, 512], one PSUM bank per pair
                th0 = tpool.tile([128, 512], f16, tag="th0")
                th1 = tpool.tile([128, 512], f16, tag="th1")
                t8m = tpool.tile([128, 2, 512], f8, tag="t8m")
                tl8 = tpool.tile([128, 2, 512], f8, tag="tl8")
                if kp == 0:
                    # spread the startup burst across idle engine queues so
                    # descriptor issue (~650ns each) doesn't serialize on Sync
                    nc.scalar.dma_start(th0[:], THX.ap()[2 * kp])
                    nc.scalar.dma_start(th1[:], THX.ap()[2 * kp + 1])
                    nc.gpsimd.dma_start(t8m[:], T8M.ap()[kp])
                    nc.sync.dma_start(a2p[:], A2P.ap())
                    nc.gpsimd.dma_start(tl8[:], TL8.ap()[kp])
                    nc.sync.dma_start(a1p[:], A1P.ap())
                else:
                    nc.sync.dma_start(th0[:], THX.ap()[2 * kp])
                    nc.sync.dma_start(th1[:], THX.ap()[2 * kp + 1])
                    nc.sync.dma_start(t8m[:], T8M.ap()[kp])
                    nc.sync.dma_start(tl8[:], TL8.ap()[kp])
                if kp == 0:
                    # v2 split over the scalar/gpsimd queues (idle until the
                    # epilogue); must land before k=0's affine_mul_reduce
                    for t in range(NBT):
                        eng = nc.scalar if t % 2 == 0 else nc.gpsimd
                        eng.dma_start(v2[t][:], V2N.ap()[t * 128:(t + 1) * 128, :])
                for t in range(NBT):
                    bsl = slice(t * 128, (t + 1) * 128)
                    ps = psb.tile([128, 2 * D], f32, tag="ps")
                    # hi,hi then DR,DR: rapid fp16<->DoubleRow alternation
                    # intermittently faults the exec unit (probed on HW);
                    # paired ordering is stable at ~64ns/tile LDW cost
                    nc.tensor.matmul(ps[:], a1[0][:, bsl], th0[:],
                                     start=True, stop=False)
                    nc.tensor.matmul(ps[:], a1[1][:, bsl], th1[:],
                                     start=False, stop=False)
                    nc.tensor.matmul(ps[:], a2p[:, :, bsl], t8m[:],
                                     perf_mode=DR, start=False, stop=False)
                    nc.tensor.matmul(ps[:], a1p[:, :, bsl], tl8[:],
                                     perf_mode=DR, start=False, stop=True)
                    sc = scr.tile([128, D], f32, tag="sc")
                    nc.vector.affine_mul_reduce(
                        out=sc[:], accum_out=bil[t][:, k:k + 1],
                        in0=ps[:, 0:D], in1=v2[t][:], scale=UNSCALE, bias=0.0,
                    )
                    sc2 = scr.tile([128, D], f32, tag="sc2")
                    nc.vector.affine_mul_reduce(
                        out=sc2[:], accum_out=bil[t][:, k + 1:k + 2],
                        in0=ps[:, D:2 * D], in1=v2[t][:], scale=UNSCALE, bias=0.0,
                    )

            # epilogue: out = tanh(bil + ff)
            for t in range(NBT):
                pre = scr.tile([128, K], f32, tag="pre")
                nc.vector.scalar_tensor_tensor(
                    pre[:], ffp[:, t * K:(t + 1) * K], 2.0 ** -11, bil[t][:],
                    mybir.AluOpType.mult, mybir.AluOpType.add,
                )
                ot = scr.tile([128, K], f32, tag="ot")
                nc.scalar.activation(
                    ot[:], pre[:], mybir.ActivationFunctionType.Tanh,
                )
                nc.sync.dma_start(OUT.ap()[t * 128:(t + 1) * 128, :], ot[:])

    nc.compile()
    return nc


def _f8(x):
    import ml_dtypes
    return np.clip(x, -F8MAX, F8MAX).astype(ml_dtypes.float8_e4m3)


def _prep_inputs(V1, V2, T, W, b):
    V1 = np.asarray(V1, np.float32)
    V2 = np.asarray(V2, np.float32)
    T = np.asarray(T, np.float32)
    W = np.asarray(W, np.float32)
    b = np.asarray(b, np.float32)

    Ts = T * np.float32(SCALE)
    TH = Ts.astype(np.float16)
    TLf = Ts - TH.astype(np.float32)
    THf = TH.astype(np.float32)

    # THX[2kp+c, p, j*256+e] = TH[2kp+j, c*128+p, e]
    THX = np.ascontiguousarray(
        TH.reshape(NKP, 2, 2, 128, 256).transpose(0, 2, 3, 1, 4)
        .reshape(2 * NKP, 128, 512))
    # T8M[kp, p, s, j*256+e] = f8(TH[2kp+j, s*128+p, e] / 64)
    T8M = _f8((THf / np.float32(F8S))
              .reshape(NKP, 2, 2, 128, 256).transpose(0, 3, 2, 1, 4)
              .reshape(NKP, 128, 2, 512))
    TL8 = _f8((TLf * np.float32(F8S))
              .reshape(NKP, 2, 2, 128, 256).transpose(0, 3, 2, 1, 4)
              .reshape(NKP, 128, 2, 512))

    V1s = V1 * np.float32(SCALE)
    A1f = V1s.astype(np.float16)
    A2f = V1s - A1f.astype(np.float32)  # fp32 residue [B, D]

    # ff with sum(b) folded in: CT gets a ones-row block, W a sum_b row.
    CTf = np.concatenate([V1, V2], axis=1)  # [B, 512]
    sum_b = np.float32(b.sum(dtype=np.float64))
    Wx = np.zeros((NC_FF * 128, K), dtype=np.float32)
    Wx[:512] = W
    Wx[512, :] = sum_b
    WHf = Wx.astype(np.float16)
    WLf = (Wx - WHf.astype(np.float32)).astype(np.float16)

    in_maps = []
    for c in range(NCORES):
        sl = slice(c * BS, (c + 1) * BS)
        A1T = np.ascontiguousarray(A1f[sl].T)             # [D, BS] fp16
        A2T = A2f[sl].T.astype(np.float32)                # [D, BS]
        A2P = _f8((A2T * np.float32(F8S))
                  .reshape(2, 128, BS).transpose(1, 0, 2))
        A1P = _f8((A1T.astype(np.float32) / np.float32(F8S))
                  .reshape(2, 128, BS).transpose(1, 0, 2))
        CTx = np.zeros((NC_FF * 128, BS), dtype=np.float32)
        CTx[:512] = CTf[sl].T
        CTx[512, :] = 1.0
        CTx *= np.float32(SCALE)
        CTHf = CTx.astype(np.float16)
        CTLf = (CTx - CTHf.astype(np.float32)).astype(np.float16)
        in_maps.append({
            "A1": A1T,
            "A2P": np.ascontiguousarray(A2P),
            "A1P": np.ascontiguousarray(A1P),
            "THX": THX,
            "T8M": T8M,
            "TL8": TL8,
            "V2N": V2[sl],
            "CTH": CTHf,
            "CTL": CTLf,
            "WH": WHf,
            "WL": WLf,
        })
    return in_maps


def kernel(V1, V2, T, W, b):
    if "nc" not in _NC_CACHE:
        _NC_CACHE["nc"] = _build()
    nc = _NC_CACHE["nc"]
    in_maps = _prep_inputs(V1, V2, T, W, b)
    res = bass_utils.run_bass_kernel_spmd(nc, in_maps, core_ids=list(range(NCORES)))
    return np.concatenate([r["OUT"] for r in res.results], axis=0)


# revision 12
# speedup vs baseline: 1.0013x; 1.0013x over previous
"""TRN2 Bass kernel for nn_BilinearTensorProduct.

  out = tanh(concat(V1,V2) @ W + einsum('bd,kde,be->bk', V1, T, V2) + sum(b))
  B=8192, D=256, K=64.  Data-parallel: batch sharded 8 ways, T/W/b replicated.

Bilinear path: per (k-pair, b-tile), one PSUM group of 4 matmuls — two fp16
hi*hi chunks [128d x 128b] @ [128d x 512e] plus two fp8-e4m3 DoubleRow
correction matmuls (lo_V*hi_T and hi_V*lo_T, contraction 256 each at 0.5
cyc/row).  All operands are pre-scaled on host so every product lands in PSUM
at scale 2^22 (fp16 side: 2^11 per factor; fp8 side: the same products
rebalanced by 2^+-6 so values fit e4m3 range).  The dropped lo*lo term is
~2^-23 relative.  After the group, one fused DVE affine_mul_reduce per k
multiplies by V2 and row-reduces into the per-tile result, folding the 2^-22
unscale into its scale slot.  MM order hi,DR,hi,DR keeps every LDWEIGHTS
hidden under the preceding matmul.  The feedforward path stays a 3-pass fp16
split matmul with sum(b) folded in as an extra contraction block (ones-row in
CT, sum_b-row in W).  tanh runs on the scalar engine.
"""

import numpy as np
from contextlib import ExitStack

import concourse.bacc as bacc
import concourse.mybir as mybir
from concourse import tile
from concourse import bass_utils

B, D, K = 8192, 256, 64
NCORES = 8
BS = B // NCORES          # 1024 batch rows per core
NBT = BS // 128           # 8 b-tiles of 128 rows
NKP = K // 2              # 32 k-pairs
NC_FF = 5                 # ff contraction chunks: 4 real + 1 bias block
SCALE = 2048.0            # 2^11 pre-scale for fp16 hi/lo split
UNSCALE = 2.0 ** -22      # undo SCALE^2 after the matmul
F8S = 64.0                # fp8 rebalance factor (2^6)
F8MAX = 240.0             # e4m3 max finite; clip before cast

f32 = mybir.dt.float32
f16 = mybir.dt.float16
f8 = mybir.dt.float8e4

_NC_CACHE = {}


def _build(n_k=K):
    nc = bacc.Bacc("TRN2", target_bir_lowering=False, debug=False)
    A1 = nc.dram_tensor("A1", [D, BS], f16, kind="ExternalInput")
    A2P = nc.dram_tensor("A2P", [128, 2, BS], f8, kind="ExternalInput")
    A1P = nc.dram_tensor("A1P", [128, 2, BS], f8, kind="ExternalInput")
    THX = nc.dram_tensor("THX", [2 * NKP, 128, 512], f16, kind="ExternalInput")
    T8M = nc.dram_tensor("T8M", [NKP, 128, 2, 512], f8, kind="ExternalInput")
    TL8 = nc.dram_tensor("TL8", [NKP, 128, 2, 512], f8, kind="ExternalInput")
    V2N = nc.dram_tensor("V2N", [BS, D], f32, kind="ExternalInput")
    CTH = nc.dram_tensor("CTH", [NC_FF * 128, BS], f16, kind="ExternalInput")
    CTL = nc.dram_tensor("CTL", [NC_FF * 128, BS], f16, kind="ExternalInput")
    WH = nc.dram_tensor("WH", [NC_FF * 128, K], f16, kind="ExternalInput")
    WL = nc.dram_tensor("WL", [NC_FF * 128, K], f16, kind="ExternalInput")
    OUT = nc.dram_tensor("OUT", [BS, K], f32, kind="ExternalOutput")

    DR = mybir.MatmulPerfMode.DoubleRow

    with tile.TileContext(nc) as tc:
        with ExitStack() as ctx:
            const = ctx.enter_context(tc.tile_pool(name="const", bufs=1))
            tpool = ctx.enter_context(tc.tile_pool(name="tpool", bufs=4))
            psb = ctx.enter_context(tc.tile_pool(name="psb", bufs=7, space="PSUM"))
            psff = ctx.enter_context(tc.tile_pool(name="psff", bufs=1, space="PSUM"))
            scr = ctx.enter_context(tc.tile_pool(name="scr", bufs=3))

            a1 = [const.tile([128, BS], f16, name=f"a1_{c}", tag=f"a1_{c}")
                  for c in range(2)]
            a2p = const.tile([128, 2, BS], f8, name="a2p", tag="a2p")
            a1p = const.tile([128, 2, BS], f8, name="a1p", tag="a1p")
            v2 = [const.tile([128, D], f32, name=f"v2_{t}", tag=f"v2_{t}")
                  for t in range(NBT)]
            cth = [const.tile([128, BS], f16, name=f"cth_{c}", tag=f"cth_{c}")
                   for c in range(NC_FF)]
            ctl = [const.tile([128, BS], f16, name=f"ctl_{c}", tag=f"ctl_{c}")
                   for c in range(NC_FF)]
            wth = [const.tile([128, K], f16, name=f"wth_{c}", tag=f"wth_{c}")
                   for c in range(NC_FF)]
            wtl = [const.tile([128, K], f16, name=f"wtl_{c}", tag=f"wtl_{c}")
                   for c in range(NC_FF)]
            bil = [const.tile([128, K], f32, name=f"bil_{t}", tag=f"bil_{t}")
                   for t in range(NBT)]

            for c in range(2):
                nc.sync.dma_start(a1[c][:], A1.ap()[c * 128:(c + 1) * 128, :])

            ffp = psff.tile([128, NBT * K], f32, name="ffp", tag="ffp")

            # PE warm-up: ~3us of throwaway matmuls on a memset scratch tile
            # run during the DMA preamble, so HAM is at 8/8 and the PE is
            # draining its queue when the first real operands land.
            warm = const.tile([128, 256], f16, name="warm", tag="warm")
            nc.any.memset(warm[:], 0.0)
            for i in range(28):
                wp = psb.tile([128, 2 * D], f32, tag="ps")
                nc.tensor.matmul(wp[:, 0:128], warm[:, 0:128], warm[:, 128:256],
                                 start=True, stop=True)

            if n_k < K:  # reduced builds (sim) leave columns unwritten
                for t in range(NBT):
                    nc.any.memset(bil[t][:], 0.0)
            assert n_k % 2 == 0, "k loop is paired"
            n_kp = n_k // 2
            kp_ff = 4 if n_kp > 4 else max(n_kp - 1, 0)
            for kp in range(n_kp):
                k = 2 * kp
                # spread the 2.6MB CT/W burst over kp=1..3 so it doesn't
                # starve the T-tile stream
                if n_kp > 4:
                    ct_sched = {1: [0, 1], 2: [2, 3], 3: [4]}.get(kp, [])
                else:
                    ct_sched = list(range(NC_FF)) if kp == min(1, n_kp - 1) else []
                for c in ct_sched:
                    nc.sync.dma_start(cth[c][:], CTH.ap()[c * 128:(c + 1) * 128, :])
                    nc.sync.dma_start(ctl[c][:], CTL.ap()[c * 128:(c + 1) * 128, :])
                    nc.sync.dma_start(wth[c][:], WH.ap()[c * 128:(c + 1) * 128, :])
                    nc.sync.dma_start(wtl[c][:], WL.ap()[c * 128:(c + 1) * 128, :])
                if kp == kp_ff:
                    # feedforward (+sum(b) bias block) as fp16 hi/lo 3-pass;
                    # CT side is pre-scaled 2^11, unscaled in the epilogue
                    for t in range(NBT):
                        passes = [(cth, wth), (cth, wtl), (ctl, wth)]
                        n_mm = len(passes) * NC_FF
                        i_mm = 0
                        for cs, ws in passes:
                            for c in range(NC_FF):
                                nc.tensor.matmul(
                                    ffp[:, t * K:(t + 1) * K],
                                    cs[c][:, t * 128:(t + 1) * 128],
                                    ws[c][:],
                                    start=(i_mm == 0), stop=(i_mm == n_mm - 1),
                                )
                                i_mm += 1
                # two k's side by side: rhs [128, 512], one PSUM bank per pair
                th0 = tpool.tile([128, 512], f16, tag="th0")
                th1 = tpool.tile([128, 512], f16, tag="th1")
                t8m = tpool.tile([128, 2, 512], f8, tag="t8m")
                tl8 = tpool.tile([128, 2, 512], f8, tag="tl8")
                if kp == 0:
                    # spread the startup burst across idle engine queues so
                    # descriptor issue (~650ns each) doesn't serialize on Sync
                    nc.scalar.dma_start(th0[:], THX.ap()[2 * kp])
                    nc.scalar.dma_start(th1[:], THX.ap()[2 * kp + 1])
                    nc.gpsimd.dma_start(t8m[:], T8M.ap()[kp])
                    nc.sync.dma_start(a2p[:], A2P.ap())
                    nc.gpsimd.dma_start(tl8[:], TL8.ap()[kp])
                    nc.sync.dma_start(a1p[:], A1P.ap())
                else:
                    nc.sync.dma_start(th0[:], THX.ap()[2 * kp])
                    nc.sync.dma_start(th1[:], THX.ap()[2 * kp + 1])
                    nc.sync.dma_start(t8m[:], T8M.ap()[kp])
                    nc.sync.dma_start(tl8[:], TL8.ap()[kp])
                if kp == 0:
                    # v2 split over the scalar/gpsimd queues (idle until the
                    # epilogue); must land before k=0's affine_mul_reduce
                    for t in range(NBT):
                        eng = nc.scalar if t % 2 == 0 else nc.gpsimd
                        eng.dma_start(v2[t][:], V2N.ap()[t * 128:(t + 1) * 128, :])
                for t in range(NBT):
                    bsl = slice(t * 128, (t + 1) * 128)
                    ps = psb.tile([128, 2 * D], f32, tag="ps")
                    # hi,hi then DR,DR: rapid fp16<->DoubleRow alternation
                    # intermittently faults the exec unit (probed on HW);
                    # paired ordering is stable at ~64ns/tile LDW cost
                    nc.tensor.matmul(ps[:], a1[0][:, bsl], th0[:],
                                     start=True, stop=False)
                    nc.tensor.matmul(ps[:], a1[1][:, bsl], th1[:],
                                     start=False, stop=False)
                    nc.tensor.matmul(ps[:], a2p[:, :, bsl], t8m[:],
                                     perf_mode=DR, start=False, stop=False)
                    nc.tensor.matmul(ps[:], a1p[:, :, bsl], tl8[:],
                                     perf_mode=DR, start=False, stop=True)
                    sc = scr.tile([128, D], f32, tag="sc")
                    nc.vector.affine_mul_reduce(
                        out=sc[:], accum_out=bil[t][:, k:k + 1],
                        in0=ps[:, 0:D], in1=v2[t][:], scale=UNSCALE, bias=0.0,
                    )
                    sc2 = scr.tile([128, D], f32, tag="sc2")
                    nc.vector.affine_mul_reduce(
                        out=sc2[:], accum_out=bil[t][:, k + 1:k + 2],
                        in0=ps[:, D:2 * D], in1=v2[t][:], scale=UNSCALE, bias=0.0,
                    )

            # epilogue: out = tanh(bil + ff)
            for t in range(NBT):
                pre = scr.tile([128, K], f32, tag="pre")
                nc.vector.scalar_tensor_tensor(
                    pre[:], ffp[:, t * K:(t + 1) * K], 2.0 ** -11, bil[t][:],
                    mybir.AluOpType.mult, mybir.AluOpType.add,
                )
                ot = scr.tile([128, K], f32, tag="ot")
                nc.scalar.activation(
                    ot[:], pre[:], mybir.ActivationFunctionType.Tanh,
                )
                # spread tail DMA issue across queues (same trick as startup)
                eng = nc.sync if t % 2 == 0 else nc.gpsimd
                eng.dma_start(OUT.ap()[t * 128:(t + 1) * 128, :], ot[:])

    nc.compile()
    return nc


def _f8(x):
    import ml_dtypes
    return np.clip(x, -F8MAX, F8MAX).astype(ml_dtypes.float8_e4m3)


def _prep_inputs(V1, V2, T, W, b):
    V1 = np.asarray(V1, np.float32)
    V2 = np.asarray(V2, np.float32)
    T = np.asarray(T, np.float32)
    W = np.asarray(W, np.float32)
    b = np.asarray(b, np.float32)

    Ts = T * np.float32(SCALE)
    TH = Ts.astype(np.float16)
    TLf = Ts - TH.astype(np.float32)
    THf = TH.astype(np.float32)

    # THX[2kp+c, p, j*256+e] = TH[2kp+j, c*128+p, e]
    THX = np.ascontiguousarray(
        TH.reshape(NKP, 2, 2, 128, 256).transpose(0, 2, 3, 1, 4)
        .reshape(2 * NKP, 128, 512))
    # T8M[kp, p, s, j*256+e] = f8(TH[2kp+j, s*128+p, e] / 64)
    T8M = _f8((THf / np.float32(F8S))
              .reshape(NKP, 2, 2, 128, 256).transpose(0, 3, 2, 1, 4)
              .reshape(NKP, 128, 2, 512))
    TL8 = _f8((TLf * np.float32(F8S))
              .reshape(NKP, 2, 2, 128, 256).transpose(0, 3, 2, 1, 4)
              .reshape(NKP, 128, 2, 512))

    V1s = V1 * np.float32(SCALE)
    A1f = V1s.astype(np.float16)
    A2f = V1s - A1f.astype(np.float32)  # fp32 residue [B, D]

    # ff with sum(b) folded in: CT gets a ones-row block, W a sum_b row.
    CTf = np.concatenate([V1, V2], axis=1)  # [B, 512]
    sum_b = np.float32(b.sum(dtype=np.float64))
    Wx = np.zeros((NC_FF * 128, K), dtype=np.float32)
    Wx[:512] = W
    Wx[512, :] = sum_b
    WHf = Wx.astype(np.float16)
    WLf = (Wx - WHf.astype(np.float32)).astype(np.float16)

    in_maps = []
    for c in range(NCORES):
        sl = slice(c * BS, (c + 1) * BS)
        A1T = np.ascontiguousarray(A1f[sl].T)             # [D, BS] fp16
        A2T = A2f[sl].T.astype(np.float32)                # [D, BS]
        A2P = _f8((A2T * np.float32(F8S))
                  .reshape(2, 128, BS).transpose(1, 0, 2))
        A1P = _f8((A1T.astype(np.float32) / np.float32(F8S))
                  .reshape(2, 128, BS).transpose(1, 0, 2))
        CTx = np.zeros((NC_FF * 128, BS), dtype=np.float32)
        CTx[:512] = CTf[sl].T
        CTx[512, :] = 1.0
        CTx *= np.float32(SCALE)
        CTHf = CTx.astype(np.float16)
        CTLf = (CTx - CTHf.astype(np.float32)).astype(np.float16)
        in_maps.append({
            "A1": A1T,
            "A2P": np.ascontiguousarray(A2P),
            "A1P": np.ascontiguousarray(A1P),
            "THX": THX,
            "T8M": T8M,
            "TL8": TL8,
            "V2N": V2[sl],
            "CTH": CTHf,
            "CTL": CTLf,
            "WH": WHf,
            "WL": WLf,
        })
    return in_maps


def kernel(V1, V2, T, W, b):
    if "nc" not in _NC_CACHE:
        _NC_CACHE["nc"] = _build()
    nc = _NC_CACHE["nc"]
    in_maps = _prep_inputs(V1, V2, T, W, b)
    res = bass_utils.run_bass_kernel_spmd(nc, in_maps, core_ids=list(range(NCORES)))
    return np.concatenate([r["OUT"] for r in res.results], axis=0)
